# revision 1
# baseline (speedup 1.0000x reference)
"""Trainium2 Bass kernel for ragged-sequence gather:

    out[pid] = verified_id[num_draft_tokens * pid + accept_lens[pid] - 1]

with BS = 2_097_152 groups, num_draft_tokens = 16, verified_id fp32 of
shape [BS*16], accept_lens int64 of shape [BS] with values in [1, 16].

Strategy (8 NeuronCores, batch-sharded):
  - Core c owns groups [c*BS/8, (c+1)*BS/8): a contiguous 16 MiB slice of
    verified_id, a 1 MiB (int32) slice of accept_lens, and writes a 1 MiB
    output slice.  Fully local, no collectives.
  - On-chip, verified data is streamed as [128, F] tiles (each group of 16
    lies contiguously inside a partition row).  A custom DVE op (SEL16)
    computes  prod[p,g,k] = V[p,g,k] * (k+1 == lens[p,g])  in a single 1x
    pass using the DVE's Idx/PageIdx hardware counters (page size 16), with
    lens broadcast via a stride-0 access pattern - no iota constant, no
    mask materialization.  A segmented tensor_reduce(add) then collapses
    each group of 16 (exact: 15 zeros + the selected value), and the result
    is DMA'd out.
"""

import sys

import numpy as np

if "/opt/trn_rl_repo" not in sys.path:
    sys.path.insert(0, "/opt/trn_rl_repo")

P = 128
ND = 16
BS = 2_097_152
N_CORES = 8
G_CORE = BS // N_CORES              # groups per core = 262144
FD_CORE = G_CORE * ND // P          # fp32 elems per partition = 32768
G_P = G_CORE // P                   # groups per partition = 2048

_SEL16_NAME = "ANT_SELECT16_V1"
_sel16_op = None


def _get_sel16():
    """Build + register the custom DVE op at runtime (appended to OPS).

    body: out[k] = select(Idx + (1 - 16*page) == Src1, Src0, 0)
    With in0 = V as [P, S, 16] and in1 = lens (f32) broadcast [P, S, 16],
    Idx is the global element counter and PageIdx(One, s0=-16) holds
    1 - 16*s within page s, so Idx + pg = (k_within_page + 1) in [1, 16].
    """
    global _sel16_op
    if _sel16_op is not None:
        return _sel16_op
    from concourse import dve_ops as dvo
    from concourse.dve_spec import (
        Spec, Src0, Src1, C0, Zero, One, eq, select, PageIdx, Idx, lower,
    )
    from concourse.dve_uop import DveOpSpec

    pg = PageIdx(One, C0)            # 1 + s*c0, call with s0 = -16.0
    body = select(eq(Idx + pg, Src1), Src0, Zero)

    def _ref(in0, in1, c0, c1, c2):
        a = np.asarray(in0, np.float32)
        l = np.asarray(in1, np.float32)
        p = a.shape[0]
        a3 = a.reshape(p, -1, ND)
        l3 = np.broadcast_to(l.reshape(p, -1, ND) if l.size == a.size
                             else l.reshape(p, -1, 1), a3.shape)
        s = a3.shape[1]
        gidx = np.arange(s * ND, dtype=np.float32).reshape(1, s, ND)
        pgv = 1.0 + np.arange(s, dtype=np.float32).reshape(1, s, 1) * float(c0)
        mask = (gidx + pgv) == l3
        return np.where(mask, a3, np.float32(0.0)).reshape(a.shape)

    spec = Spec(body=body, reference=_ref)
    shas = {}
    for ver in ("v3", "v4"):
        try:
            uops = lower(spec, ver=ver)
            shas[ver] = DveOpSpec(
                name=_SEL16_NAME, opcode=1, uops=uops, rd1_en=True
            ).sha(ver)
        except Exception:
            pass

    op = dvo.DveOp(_SEL16_NAME, spec, subdim=True, uops_sha=shas)
    if _SEL16_NAME not in dvo._SUB_OPCODE_FOR_NAME:
        dvo.OPS.append(op)
        row = dvo._CUSTOM_DVE_ROW_BASE + len(dvo.OPS) - 1
        assert row < 0x20
        dvo._SUB_OPCODE_FOR_NAME[_SEL16_NAME] = row
        dvo.CUSTOM_DVE_SPECS[_SEL16_NAME] = spec
    _sel16_op = op
    return op


def build_bass(fd_p=FD_CORE, nt=0, ramp=2, tail_ramp=0, gp=0, lens_cast=0, sched=0, vb=4, ldma=1):
    """Build the per-core Bass program.

    fd_p: total fp32 elements per partition (divisible by nt*16)
    nt:   number of full-size tiles the bulk is split into
    ramp: split the first tile into `ramp` sub-tiles for a faster pipeline
          warm-up (0/1 = disabled)
    tail_ramp: split the last tile into `tail_ramp` sub-tiles so the final
          output DMA shrinks (0/1 = disabled)
    gp:   number of full-size tiles whose select is computed on GPSIMD via
          the stock mask pipeline (ACT expansion + eq + mult) instead of the
          DVE custom op, to offload the DVE bottleneck
    """
    import concourse.bacc as bacc
    import concourse.mybir as mybir
    import ml_dtypes
    from concourse.tile import TileContext

    f32 = mybir.dt.float32
    i32 = mybir.dt.int32
    bf16 = mybir.dt.bfloat16

    fdt = fd_p // nt if nt else fd_p
    assert fdt % ND == 0 and fdt * (nt or 1) == fd_p
    g_p = fd_p // ND

    # tile schedule: (elem offset, elems) per partition
    if nt == 0:
        # mixed schedule: small tiles to ramp the pipeline, 4096-elem tiles
        # for the overhead-amortized steady phase, 2048 tail
        if sched == 2:
            sizes = [1024] * 2 + [2048] * 2 + [4096] * 6 + [1024] + [512] * 2
        elif sched:
            sizes = [1024] * 2 + [2048] * 3 + [4096] * 5 + [2048] * 2
        else:
            sizes = [1024] * 2 + [2048] * 2 + [4096] * 6 + [2048]
        assert sum(sizes) == fd_p
        tiles, off0 = [], 0
        for s in sizes:
            tiles.append((off0, s))
            off0 += s
    else:
        tiles = [(t * fdt, fdt) for t in range(nt)]
        if ramp and ramp > 1 and fdt % (ramp * ND) == 0:
            sub = fdt // ramp
            tiles[0:1] = [(i * sub, sub) for i in range(ramp)]
        if tail_ramp and tail_ramp > 1 and fdt % (tail_ramp * ND) == 0:
            off0 = tiles[-1][0]
            sub = fdt // tail_ramp
            tiles[-1:] = [(off0 + i * sub, sub) for i in range(tail_ramp)]

    sel16 = _get_sel16()

    nc = bacc.Bacc("TRN2", target_bir_lowering=False)

    v_d = nc.dram_tensor("v", [P, fd_p], f32, kind="ExternalInput")
    l_d = nc.dram_tensor("lens", [P, g_p], i32, kind="ExternalInput")
    o_d = nc.dram_tensor("o", [P, g_p], f32, kind="ExternalOutput")

    # which tile indices run on GPSIMD: spread through the middle fulls
    full_idx = [i for i, (_, n) in enumerate(tiles) if n == fdt]
    gp_set = set(full_idx[1:1 + gp]) if gp else set()

    iota_d = None
    if gp_set:
        iota_np = np.tile(np.arange(1, ND + 1, dtype=np.float32), fdt // ND)
        iota_np = np.ascontiguousarray(
            iota_np.astype(ml_dtypes.bfloat16).reshape(1, fdt)
        )
        iota_d = nc.inline_tensor(iota_np, name="iota1_const")

    with TileContext(nc) as tc:
        with tc.tile_pool(name="work", bufs=3) as pool:
            if gp_set:
                iota_t = pool.tile([P, fdt], bf16, tag="iota", bufs=1)
                nc.gpsimd.dma_start(
                    out=iota_t[:], in_=iota_d[0:1, :].partition_broadcast(P)
                )
            for i, (off, n) in enumerate(tiles):
                goff, gn = off // ND, n // ND
                vt = pool.tile([P, n], f32, tag=f"v{n}", bufs=(vb if n == 4096 else 2 if n <= 1024 else 3))
                nc.sync.dma_start(out=vt[:], in_=v_d[:, off:off + n])
                lt = pool.tile([P, gn], i32, tag=f"l{n}")
                # ldma=1: lens via SWDGE so it never queues behind V tiles
                # on the HWDGE rings during the ramp
                (nc.gpsimd if ldma else nc.sync).dma_start(
                    out=lt[:], in_=l_d[:, goff:goff + gn])

                if lens_cast:
                    # int32 -> fp32 cast on the (otherwise idle) ACT engine
                    lf = pool.tile([P, gn], f32, tag=f"lf{n}")
                    nc.scalar.copy(out=lf[:], in_=lt[:])
                else:
                    # DVE read port converts int32 -> fp32 internally
                    lf = lt

                prod = pool.tile([P, n], f32, tag=f"prod{n}", bufs=2)
                if i in gp_set:
                    # GPSIMD pipeline: ACT expands lens to bf16, GPSIMD does
                    # eq + mult (frees the DVE for other tiles)
                    lexp = pool.tile([P, n], bf16, tag="lexp", bufs=2)
                    nc.scalar.copy(
                        out=lexp[:].rearrange("p (g k) -> p g k", k=ND),
                        in_=lf[:, :, None].to_broadcast([P, gn, ND]),
                    )
                    # eq on DVE (2x bf16), mult on GPSIMD (Pool rejects the
                    # BITVEC is_equal opcode but supports ARITH mult)
                    mask = pool.tile([P, n], bf16, tag="mask", bufs=2)
                    nc.vector.tensor_tensor(
                        out=mask[:], in0=lexp[:], in1=iota_t[:],
                        op=mybir.AluOpType.is_equal,
                    )
                    nc.gpsimd.tensor_tensor(
                        out=prod[:], in0=mask[:], in1=vt[:],
                        op=mybir.AluOpType.mult,
                    )
                else:
                    nc.vector._custom_dve(
                        sel16,
                        out=prod[:].rearrange("p (g k) -> p g k", k=ND),
                        in0=vt[:].rearrange("p (g k) -> p g k", k=ND),
                        in1=lf[:, :, None].to_broadcast([P, gn, ND]),
                        s0=-float(ND),
                    )

                ot = pool.tile([P, gn], f32, tag=f"o{n}")
                nc.vector.tensor_reduce(
                    out=ot[:],
                    in_=prod[:].rearrange("p (g k) -> p g k", k=ND),
                    axis=mybir.AxisListType.X,
                    op=mybir.AluOpType.add,
                )
                nc.sync.dma_start(out=o_d[:, goff:goff + gn], in_=ot[:])
    if not nc.is_finalized():
        nc.finalize()
    return nc


_CACHE = {}


def _get_nc(**kw):
    key = tuple(sorted(kw.items()))
    if key not in _CACHE:
        _CACHE[key] = build_bass(**kw)
    return _CACHE[key]


def kernel(verified_id, accept_lens, num_draft_tokens, **run_kw):
    from concourse.bass_utils import run_bass_kernel_spmd

    assert int(num_draft_tokens) == ND
    v = np.ascontiguousarray(np.asarray(verified_id, dtype=np.float32))
    lens = np.asarray(accept_lens)
    assert v.shape == (BS * ND,) and lens.shape == (BS,)
    l32 = np.ascontiguousarray(lens.astype(np.int32))

    v3 = v.reshape(N_CORES, P, FD_CORE)
    l3 = l32.reshape(N_CORES, P, G_P)

    nc = _get_nc()
    in_maps = [{"v": v3[c], "lens": l3[c]} for c in range(N_CORES)]
    res = run_bass_kernel_spmd(nc, in_maps, core_ids=list(range(N_CORES)), **run_kw)
    out = np.stack([res.results[c]["o"] for c in range(N_CORES)])
    ret = out.reshape(-1)
    if run_kw:
        return ret, res
    return ret



# revision 10
# speedup vs baseline: 1.0566x; 1.0566x over previous
"""Trainium2 Bass kernel for ragged-sequence gather:

    out[pid] = verified_id[num_draft_tokens * pid + accept_lens[pid] - 1]

with BS = 2_097_152 groups, num_draft_tokens = 16, verified_id fp32 of
shape [BS*16], accept_lens int64 of shape [BS] with values in [1, 16].

Strategy (8 NeuronCores, batch-sharded):
  - Core c owns groups [c*BS/8, (c+1)*BS/8): a contiguous 16 MiB slice of
    verified_id, a 1 MiB (int32) slice of accept_lens, and writes a 1 MiB
    output slice.  Fully local, no collectives.
  - On-chip, verified data is streamed as [128, F] tiles (each group of 16
    lies contiguously inside a partition row).  A custom DVE op (SEL16)
    computes  prod[p,g,k] = V[p,g,k] * (k+1 == lens[p,g])  in a single 1x
    pass using the DVE's Idx/PageIdx hardware counters (page size 16), with
    lens broadcast via a stride-0 access pattern - no iota constant, no
    mask materialization.  A segmented tensor_reduce(add) then collapses
    each group of 16 (exact: 15 zeros + the selected value), and the result
    is DMA'd out.
"""

import sys

import numpy as np

if "/opt/trn_rl_repo" not in sys.path:
    sys.path.insert(0, "/opt/trn_rl_repo")

P = 128
ND = 16
BS = 2_097_152
N_CORES = 8
G_CORE = BS // N_CORES              # groups per core = 262144
FD_CORE = G_CORE * ND // P          # fp32 elems per partition = 32768
G_P = G_CORE // P                   # groups per partition = 2048

_SEL16_NAME = "ANT_SELECT16_V1"
_sel16_op = None


def _get_sel16():
    """Build + register the custom DVE op at runtime (appended to OPS).

    body: out[k] = select(Idx + (1 - 16*page) == Src1, Src0, 0)
    With in0 = V as [P, S, 16] and in1 = lens (f32) broadcast [P, S, 16],
    Idx is the global element counter and PageIdx(One, s0=-16) holds
    1 - 16*s within page s, so Idx + pg = (k_within_page + 1) in [1, 16].
    """
    global _sel16_op
    if _sel16_op is not None:
        return _sel16_op
    from concourse import dve_ops as dvo
    from concourse.dve_spec import (
        Spec, Src0, Src1, C0, Zero, One, eq, select, PageIdx, Idx, lower,
    )
    from concourse.dve_uop import DveOpSpec

    pg = PageIdx(One, C0)            # 1 + s*c0, call with s0 = -16.0
    body = select(eq(Idx + pg, Src1), Src0, Zero)

    def _ref(in0, in1, c0, c1, c2):
        a = np.asarray(in0, np.float32)
        l = np.asarray(in1, np.float32)
        p = a.shape[0]
        a3 = a.reshape(p, -1, ND)
        l3 = np.broadcast_to(l.reshape(p, -1, ND) if l.size == a.size
                             else l.reshape(p, -1, 1), a3.shape)
        s = a3.shape[1]
        gidx = np.arange(s * ND, dtype=np.float32).reshape(1, s, ND)
        pgv = 1.0 + np.arange(s, dtype=np.float32).reshape(1, s, 1) * float(c0)
        mask = (gidx + pgv) == l3
        return np.where(mask, a3, np.float32(0.0)).reshape(a.shape)

    spec = Spec(body=body, reference=_ref)
    shas = {}
    for ver in ("v3", "v4"):
        try:
            uops = lower(spec, ver=ver)
            shas[ver] = DveOpSpec(
                name=_SEL16_NAME, opcode=1, uops=uops, rd1_en=True
            ).sha(ver)
        except Exception:
            pass

    op = dvo.DveOp(_SEL16_NAME, spec, subdim=True, uops_sha=shas)
    if _SEL16_NAME not in dvo._SUB_OPCODE_FOR_NAME:
        dvo.OPS.append(op)
        row = dvo._CUSTOM_DVE_ROW_BASE + len(dvo.OPS) - 1
        assert row < 0x20
        dvo._SUB_OPCODE_FOR_NAME[_SEL16_NAME] = row
        dvo.CUSTOM_DVE_SPECS[_SEL16_NAME] = spec
    _sel16_op = op
    return op


_SEL16M_NAME = "ANT_SELECT16_MAX_V1"
_sel16m_op = None


def _get_sel16m():
    """select(k+1 == lens, V, -FLT_MAX): like SEL16 but the filler is -BIG
    so the segmented 16->1 reduce can be MAX (which, unlike ADD, permits a
    bf16 output => DVE 2x perf-mode eligible)."""
    global _sel16m_op
    if _sel16m_op is not None:
        return _sel16m_op
    from concourse import dve_ops as dvo
    from concourse.dve_spec import (
        Spec, Src0, Src1, C0, Zero, One, MaxNeg, eq, select, PageIdx, Idx,
        lower,
    )
    from concourse.dve_uop import DveOpSpec

    pg = PageIdx(One, C0)            # 1 + s*c0, call with s0 = -16.0
    body = select(eq(Idx + pg, Src1), Src0, MaxNeg)

    _NEG = np.float32(-3.4028235e38)

    def _ref(in0, in1, c0, c1, c2):
        a = np.asarray(in0, np.float32)
        l = np.asarray(in1, np.float32)
        p = a.shape[0]
        a3 = a.reshape(p, -1, ND)
        l3 = np.broadcast_to(l.reshape(p, -1, ND) if l.size == a.size
                             else l.reshape(p, -1, 1), a3.shape)
        s = a3.shape[1]
        gidx = np.arange(s * ND, dtype=np.float32).reshape(1, s, ND)
        pgv = 1.0 + np.arange(s, dtype=np.float32).reshape(1, s, 1) * float(c0)
        mask = (gidx + pgv) == l3
        return np.where(mask, a3, _NEG).reshape(a.shape)

    spec = Spec(body=body, reference=_ref)
    shas = {}
    for ver in ("v3", "v4"):
        try:
            uops = lower(spec, ver=ver)
            shas[ver] = DveOpSpec(
                name=_SEL16M_NAME, opcode=1, uops=uops, rd1_en=True
            ).sha(ver)
        except Exception:
            pass

    op = dvo.DveOp(_SEL16M_NAME, spec, subdim=True, uops_sha=shas)
    if _SEL16M_NAME not in dvo._SUB_OPCODE_FOR_NAME:
        dvo.OPS.append(op)
        row = dvo._CUSTOM_DVE_ROW_BASE + len(dvo.OPS) - 1
        assert row < 0x20
        dvo._SUB_OPCODE_FOR_NAME[_SEL16M_NAME] = row
        dvo.CUSTOM_DVE_SPECS[_SEL16M_NAME] = spec
    _sel16m_op = op
    return op


def build_v3(fd_p=FD_CORE, sched=2, vb=6, tree=1, lens_u8=1, out_bf16=1,
             dve_reduce_tiles=()):
    """v3: max-select pipeline.

    - one upfront lens DMA (u8) + one-time ACT cast to f32
    - DVE custom op SEL16M: prod = select(k+1==lens, V, -BIG) -> bf16
    - Pool tensor_tensor(max) folds 16->8 (tree=1) or 16->4 (tree=2)
    - DVE tensor_reduce(max, X) collapses the rest -> bf16 out
    - bf16 output DMA; host converts back to f32
    dve_reduce_tiles: tile indices that skip the Pool fold (direct reduce16)
    """
    import concourse.bacc as bacc
    import concourse.mybir as mybir
    from concourse.tile import TileContext

    f32 = mybir.dt.float32
    i32 = mybir.dt.int32
    u8 = mybir.dt.uint8
    bf16 = mybir.dt.bfloat16

    g_p = fd_p // ND

    if sched == 0:
        sizes = [1024] * 2 + [2048] * 2 + [4096] * 6 + [2048]
    elif sched == 1:
        sizes = [512] * 2 + [1024] + [2048] * 2 + [4096] * 6 + [1024] + [512] * 2 + [1024]
    elif sched == 2:
        sizes = [1024] * 2 + [2048] * 2 + [4096] * 6 + [1024] + [512] * 2
    assert sum(sizes) == fd_p, sum(sizes)
    tiles, off0 = [], 0
    for s in sizes:
        tiles.append((off0, s))
        off0 += s

    sel16m = _get_sel16m()

    nc = bacc.Bacc("TRN2", target_bir_lowering=False)

    v_d = nc.dram_tensor("v", [P, fd_p], f32, kind="ExternalInput")
    l_d = nc.dram_tensor("lens", [P, g_p], u8 if lens_u8 else i32,
                         kind="ExternalInput")
    odt = bf16 if out_bf16 else f32
    o_d = nc.dram_tensor("o", [P, g_p], odt, kind="ExternalOutput")

    with TileContext(nc) as tc:
        with tc.tile_pool(name="work", bufs=3) as pool:
            lraw = pool.tile([P, g_p], u8 if lens_u8 else i32, tag="lraw",
                             bufs=1)
            nc.sync.dma_start(out=lraw[:], in_=l_d[:, :])
            if lens_u8:
                lens_t = pool.tile([P, g_p], f32, tag="lensf", bufs=1)
                nc.scalar.copy(out=lens_t[:], in_=lraw[:])
            else:
                lens_t = lraw

            for i, (off, n) in enumerate(tiles):
                goff, gn = off // ND, n // ND
                vt = pool.tile([P, n], f32, tag=f"v{n}",
                               bufs=(vb if n == 4096 else 2 if n <= 1024 else 3))
                nc.sync.dma_start(out=vt[:], in_=v_d[:, off:off + n])

                prod = pool.tile([P, n], bf16, tag=f"prod{n}", bufs=2)
                nc.vector._custom_dve(
                    sel16m,
                    out=prod[:].rearrange("p (g k) -> p g k", k=ND),
                    in0=vt[:].rearrange("p (g k) -> p g k", k=ND),
                    in1=lens_t[:, goff:goff + gn, None].to_broadcast(
                        [P, gn, ND]),
                    s0=-float(ND),
                )
                p3 = prod[:].rearrange("p (g k) -> p g k", k=ND)

                if tree and i not in dve_reduce_tiles:
                    h8 = pool.tile([P, gn * 8], bf16, tag=f"h8_{n}", bufs=2)
                    h83 = h8[:].rearrange("p (g k) -> p g k", k=8)
                    nc.gpsimd.tensor_tensor(
                        out=h83, in0=p3[:, :, 0:8], in1=p3[:, :, 8:16],
                        op=mybir.AluOpType.max,
                    )
                    red_in, red_k = h83, 8
                    if tree >= 2:
                        h4 = pool.tile([P, gn * 4], bf16, tag=f"h4_{n}",
                                       bufs=2)
                        h43 = h4[:].rearrange("p (g k) -> p g k", k=4)
                        nc.gpsimd.tensor_tensor(
                            out=h43, in0=h83[:, :, 0:4], in1=h83[:, :, 4:8],
                            op=mybir.AluOpType.max,
                        )
                        red_in, red_k = h43, 4
                else:
                    red_in, red_k = p3, ND

                ot = pool.tile([P, gn], odt, tag=f"o{n}")
                nc.vector.tensor_reduce(
                    out=ot[:],
                    in_=red_in,
                    axis=mybir.AxisListType.X,
                    op=mybir.AluOpType.max,
                )
                nc.sync.dma_start(out=o_d[:, goff:goff + gn], in_=ot[:])
    if not nc.is_finalized():
        nc.finalize()
    return nc


_SELK_NAME = "ANT_SELECT_KMAJOR_V1"
_selk_op = None


def _get_selk():
    """k-major select: stream pages are k-blocks of gn elements.

    out[p, k, g] = (k+1 == lens[p, g]) ? V[p, k, g] : 0
    pg = PageIdx(One, C0) with s0=+1 holds k+1 in page k; Src1 is lens
    broadcast per page (packed innermost!).
    """
    global _selk_op
    if _selk_op is not None:
        return _selk_op
    from concourse import dve_ops as dvo
    from concourse.dve_spec import (
        Spec, Src0, Src1, C0, Zero, One, eq, select, PageIdx, lower,
    )
    from concourse.dve_uop import DveOpSpec

    pg = PageIdx(One, C0)            # 1 + s*c0, call with s0 = +1.0
    body = select(eq(pg, Src1), Src0, Zero)

    def _ref(in0, in1, c0, c1, c2):
        a = np.asarray(in0, np.float32)
        l = np.asarray(in1, np.float32)
        p = a.shape[0]
        npages = 16
        a3 = a.reshape(p, npages, -1)
        l3 = np.broadcast_to(l.reshape(p, npages, -1) if l.size == a.size
                             else l.reshape(p, 1, -1), a3.shape)
        pgv = 1.0 + np.arange(npages, dtype=np.float32).reshape(
            1, npages, 1) * float(c0)
        mask = pgv == l3
        return np.where(mask, a3, np.float32(0.0)).reshape(a.shape)

    spec = Spec(body=body, reference=_ref)
    shas = {}
    for ver in ("v3", "v4"):
        try:
            uops = lower(spec, ver=ver)
            shas[ver] = DveOpSpec(
                name=_SELK_NAME, opcode=1, uops=uops, rd1_en=True
            ).sha(ver)
        except Exception:
            pass

    op = dvo.DveOp(_SELK_NAME, spec, subdim=True, uops_sha=shas)
    if _SELK_NAME not in dvo._SUB_OPCODE_FOR_NAME:
        dvo.OPS.append(op)
        row = dvo._CUSTOM_DVE_ROW_BASE + len(dvo.OPS) - 1
        assert row < 0x20
        dvo._SUB_OPCODE_FOR_NAME[_SELK_NAME] = row
        dvo.CUSTOM_DVE_SPECS[_SELK_NAME] = spec
    _selk_op = op
    return op


V5_SCHEDS = {
    0: [1024] * 2 + [2048] * 2 + [4096] * 6 + [2048],
    2: [1024] * 2 + [2048] * 2 + [4096] * 6 + [1024] + [512] * 2,
}


def build_v5(fd_p=FD_CORE, sched=2, vb=6, lens_u8=1, pool_l1=2, pool_l2=0,
             out_f32=0):
    """v5: host-side k-major V layout per tile.

    Per tile, V arrives as [P, 16, gn] (k-major, contiguous DMA since the
    host pre-transposes). The SELK custom op zeroes non-selected entries
    (1x, unavoidable); the 16->1 fold is 4 levels of CONTIGUOUS packed
    bf16 tensor_tensor adds that run in DVE 2x mode; L1 (the big one) can
    go to Pool every pool_l1-th full tile. Output is bf16, natural order.
    """
    import concourse.bacc as bacc
    import concourse.mybir as mybir
    from concourse.tile import TileContext

    f32 = mybir.dt.float32
    i32 = mybir.dt.int32
    u8 = mybir.dt.uint8
    bf16 = mybir.dt.bfloat16

    g_p = fd_p // ND
    sizes = V5_SCHEDS[sched]
    assert sum(sizes) == fd_p, sum(sizes)
    tiles, off0 = [], 0
    for s in sizes:
        tiles.append((off0, s))
        off0 += s

    selk = _get_selk()

    nc = bacc.Bacc("TRN2", target_bir_lowering=False)

    v_d = nc.dram_tensor("v", [P, fd_p], f32, kind="ExternalInput")
    l_d = nc.dram_tensor("lens", [P, g_p], u8 if lens_u8 else i32,
                         kind="ExternalInput")
    odt = f32 if out_f32 else bf16
    o_d = nc.dram_tensor("o", [P, g_p], odt, kind="ExternalOutput")

    with TileContext(nc) as tc:
        with tc.tile_pool(name="work", bufs=3) as pool:
            lraw = pool.tile([P, g_p], u8 if lens_u8 else i32, tag="lraw",
                             bufs=1)
            nc.sync.dma_start(out=lraw[:], in_=l_d[:, :])
            if lens_u8:
                lens_t = pool.tile([P, g_p], f32, tag="lensf", bufs=1)
                nc.scalar.copy(out=lens_t[:], in_=lraw[:])
            else:
                lens_t = lraw

            nfull = 0
            for i, (off, n) in enumerate(tiles):
                goff, gn = off // ND, n // ND
                vt = pool.tile([P, n], f32, tag=f"v{n}",
                               bufs=(vb if n == 4096 else 2 if n <= 1024 else 3))
                nc.sync.dma_start(out=vt[:], in_=v_d[:, off:off + n])

                # prod, k-major contiguous (same element order as vt)
                pt = pool.tile([P, n], bf16, tag=f"pt{n}", bufs=2)
                nc.vector._custom_dve(
                    selk,
                    out=pt[:].rearrange("p (k g) -> p k g", g=gn),
                    in0=vt[:].rearrange("p (k g) -> p k g", g=gn),
                    in1=lens_t[:, None, goff:goff + gn].to_broadcast(
                        [P, ND, gn]),
                    s0=1.0,
                )

                if n == 4096:
                    nfull += 1
                l1_pool = pool_l1 and (n == 4096) and (nfull % pool_l1 == 0)
                l2_pool = pool_l2 and (n == 4096) and (nfull % pool_l2 == 0)

                h8 = pool.tile([P, n // 2], bf16, tag=f"h8_{n}", bufs=2)
                eng = nc.gpsimd if l1_pool else nc.vector
                eng.tensor_tensor(out=h8[:], in0=pt[:, 0:n // 2],
                                  in1=pt[:, n // 2:n],
                                  op=mybir.AluOpType.add)
                h4 = pool.tile([P, n // 4], bf16, tag=f"h4_{n}", bufs=2)
                eng = nc.gpsimd if l2_pool else nc.vector
                eng.tensor_tensor(out=h4[:], in0=h8[:, 0:n // 4],
                                  in1=h8[:, n // 4:n // 2],
                                  op=mybir.AluOpType.add)
                h2 = pool.tile([P, n // 8], bf16, tag=f"h2_{n}", bufs=2)
                nc.vector.tensor_tensor(out=h2[:], in0=h4[:, 0:n // 8],
                                        in1=h4[:, n // 8:n // 4],
                                        op=mybir.AluOpType.add)
                ot = pool.tile([P, gn], odt, tag=f"o{n}")
                nc.vector.tensor_tensor(out=ot[:], in0=h2[:, 0:gn],
                                        in1=h2[:, gn:2 * gn],
                                        op=mybir.AluOpType.add)
                nc.sync.dma_start(out=o_d[:, goff:goff + gn], in_=ot[:])
    if not nc.is_finalized():
        nc.finalize()
    return nc


def v5_host_transpose(v3, sched):
    """[N_CORES, P, FD_CORE] natural -> per-tile k-major layout."""
    sizes = V5_SCHEDS[sched]
    parts = []
    goff = 0
    for s in sizes:
        gn = s // ND
        chunk = v3[:, :, goff * ND:(goff + gn) * ND]
        parts.append(np.transpose(
            chunk.reshape(N_CORES, P, gn, ND), (0, 1, 3, 2)
        ).reshape(N_CORES, P, s))
        goff += gn
    return np.ascontiguousarray(np.concatenate(parts, axis=2))


def build_v4(fd_p=FD_CORE, sched=2, vb=7, lens_u8=1, pool_l1=1, pool_l2=0,
             out_f32=0):
    """v4: k-major select + contiguous bf16 add-tree (DVE 2x perf mode).

    - SEL16 (zero filler) writes prod bf16 with a TRANSPOSED k-major out AP
      (free at 1x) so every tree level folds two CONTIGUOUS packed bf16
      halves -> DVE tensor_tensor(add) runs in 2x mode (0.55 ns/elem).
    - L1 (n -> n/2) optionally on Pool to offload DVE; L2 optionally too.
    - one upfront u8 lens DMA + one-time ACT cast to f32.
    - bf16 output (host converts), ~0.4% rel err from bf16 rounding of V.
    pool_l1/pool_l2: every which-th full tile offloads that level to Pool
      (0 = never, 1 = always, 2 = every other, ...)
    """
    import concourse.bacc as bacc
    import concourse.mybir as mybir
    from concourse.tile import TileContext

    f32 = mybir.dt.float32
    i32 = mybir.dt.int32
    u8 = mybir.dt.uint8
    bf16 = mybir.dt.bfloat16

    g_p = fd_p // ND

    if sched == 0:
        sizes = [1024] * 2 + [2048] * 2 + [4096] * 6 + [2048]
    elif sched == 1:
        sizes = [512] * 2 + [1024] + [2048] * 2 + [4096] * 6 + [1024] + [512] * 2 + [1024]
    elif sched == 2:
        sizes = [1024] * 2 + [2048] * 2 + [4096] * 6 + [1024] + [512] * 2
    assert sum(sizes) == fd_p, sum(sizes)
    tiles, off0 = [], 0
    for s in sizes:
        tiles.append((off0, s))
        off0 += s

    sel16 = _get_sel16()

    nc = bacc.Bacc("TRN2", target_bir_lowering=False)

    v_d = nc.dram_tensor("v", [P, fd_p], f32, kind="ExternalInput")
    l_d = nc.dram_tensor("lens", [P, g_p], u8 if lens_u8 else i32,
                         kind="ExternalInput")
    odt = f32 if out_f32 else bf16
    o_d = nc.dram_tensor("o", [P, g_p], odt, kind="ExternalOutput")

    with TileContext(nc) as tc:
        with tc.tile_pool(name="work", bufs=3) as pool:
            lraw = pool.tile([P, g_p], u8 if lens_u8 else i32, tag="lraw",
                             bufs=1)
            nc.sync.dma_start(out=lraw[:], in_=l_d[:, :])
            if lens_u8:
                lens_t = pool.tile([P, g_p], f32, tag="lensf", bufs=1)
                nc.scalar.copy(out=lens_t[:], in_=lraw[:])
            else:
                lens_t = lraw

            nfull = 0
            for i, (off, n) in enumerate(tiles):
                goff, gn = off // ND, n // ND
                vt = pool.tile([P, n], f32, tag=f"v{n}",
                               bufs=(vb if n == 4096 else 2 if n <= 1024 else 3))
                nc.sync.dma_start(out=vt[:], in_=v_d[:, off:off + n])

                # prod, k-major: address = k*gn + g
                pt = pool.tile([P, n], bf16, tag=f"pt{n}", bufs=2)
                nc.vector._custom_dve(
                    sel16,
                    out=pt[:].rearrange("p (k g) -> p g k", g=gn),
                    in0=vt[:].rearrange("p (g k) -> p g k", k=ND),
                    in1=lens_t[:, goff:goff + gn, None].to_broadcast(
                        [P, gn, ND]),
                    s0=-float(ND),
                )

                if n == 4096:
                    nfull += 1
                l1_pool = pool_l1 and (n == 4096) and (nfull % pool_l1 == 0)
                l2_pool = pool_l2 and (n == 4096) and (nfull % pool_l2 == 0)

                h8 = pool.tile([P, n // 2], bf16, tag=f"h8_{n}", bufs=2)
                eng = nc.gpsimd if l1_pool else nc.vector
                eng.tensor_tensor(out=h8[:], in0=pt[:, 0:n // 2],
                                  in1=pt[:, n // 2:n],
                                  op=mybir.AluOpType.add)
                h4 = pool.tile([P, n // 4], bf16, tag=f"h4_{n}", bufs=2)
                eng = nc.gpsimd if l2_pool else nc.vector
                eng.tensor_tensor(out=h4[:], in0=h8[:, 0:n // 4],
                                  in1=h8[:, n // 4:n // 2],
                                  op=mybir.AluOpType.add)
                h2 = pool.tile([P, n // 8], bf16, tag=f"h2_{n}", bufs=2)
                nc.vector.tensor_tensor(out=h2[:], in0=h4[:, 0:n // 8],
                                        in1=h4[:, n // 8:n // 4],
                                        op=mybir.AluOpType.add)
                ot = pool.tile([P, gn], odt, tag=f"o{n}")
                nc.vector.tensor_tensor(out=ot[:], in0=h2[:, 0:gn],
                                        in1=h2[:, gn:2 * gn],
                                        op=mybir.AluOpType.add)
                nc.sync.dma_start(out=o_d[:, goff:goff + gn], in_=ot[:])
    if not nc.is_finalized():
        nc.finalize()
    return nc


def build_v2(fd_p=FD_CORE, sched=0, vb=5, pool_reduce=None, prod_bf16=0,
             lens_i8=0, out_bf16=0):
    """v2: single upfront lens DMA, reduce split across Pool/DVE.

    sched: tile-size schedule selector
    vb:    bufs for the 4096-elem V tiles
    pool_reduce: set of tile indices whose reduce runs on Pool (gpsimd);
          None = heuristic split (all but two mid tiles)
    prod_bf16: select writes bf16 prod; reduce reads bf16 (2x DVE mode probe)
    lens_i8: lens arrives as uint8 and is cast once on ACT
    out_bf16: result tiles are bf16 (host converts back; ~0.4% rel err)
    """
    import concourse.bacc as bacc
    import concourse.mybir as mybir
    from concourse.tile import TileContext

    f32 = mybir.dt.float32
    i32 = mybir.dt.int32
    u8 = mybir.dt.uint8
    bf16 = mybir.dt.bfloat16

    g_p = fd_p // ND

    if sched == 0:
        sizes = [1024] * 2 + [2048] * 2 + [4096] * 6 + [2048]
    elif sched == 1:
        sizes = [512] * 2 + [1024] + [2048] * 2 + [4096] * 6 + [1024] + [512] * 2 + [1024]
    elif sched == 2:
        sizes = [1024] * 2 + [2048] * 2 + [4096] * 6 + [1024] + [512] * 2
    assert sum(sizes) == fd_p, sum(sizes)
    tiles, off0 = [], 0
    for s in sizes:
        tiles.append((off0, s))
        off0 += s

    if pool_reduce is None:
        # all reduces on Pool except two full tiles kept on DVE (f ~ 0.85)
        full_idx = [i for i, (_, n) in enumerate(tiles) if n == 4096]
        pool_reduce = set(range(len(tiles))) - set(full_idx[2:4])
    pool_reduce = set(pool_reduce)

    sel16 = _get_sel16()

    nc = bacc.Bacc("TRN2", target_bir_lowering=False)

    v_d = nc.dram_tensor("v", [P, fd_p], f32, kind="ExternalInput")
    l_d = nc.dram_tensor("lens", [P, g_p], u8 if lens_i8 else i32,
                         kind="ExternalInput")
    o_d = nc.dram_tensor("o", [P, g_p], bf16 if out_bf16 else f32,
                         kind="ExternalOutput")

    pdt = bf16 if prod_bf16 else f32
    odt = bf16 if out_bf16 else f32

    with TileContext(nc) as tc:
        with tc.tile_pool(name="work", bufs=3) as pool:
            # one upfront DMA for all lens (issued first so it clears the
            # rings before the first big V tile queues behind it)
            lraw = pool.tile([P, g_p], u8 if lens_i8 else i32, tag="lraw",
                             bufs=1)
            nc.sync.dma_start(out=lraw[:], in_=l_d[:, :])
            if lens_i8:
                # one-time u8 -> f32 cast on the otherwise idle ACT engine
                lens_t = pool.tile([P, g_p], f32, tag="lensf", bufs=1)
                nc.scalar.copy(out=lens_t[:], in_=lraw[:])
            else:
                lens_t = lraw

            for i, (off, n) in enumerate(tiles):
                goff, gn = off // ND, n // ND
                vt = pool.tile([P, n], f32, tag=f"v{n}",
                               bufs=(vb if n == 4096 else 2 if n <= 1024 else 3))
                nc.sync.dma_start(out=vt[:], in_=v_d[:, off:off + n])

                prod = pool.tile([P, n], pdt, tag=f"prod{n}", bufs=2)
                nc.vector._custom_dve(
                    sel16,
                    out=prod[:].rearrange("p (g k) -> p g k", k=ND),
                    in0=vt[:].rearrange("p (g k) -> p g k", k=ND),
                    in1=lens_t[:, goff:goff + gn, None].to_broadcast(
                        [P, gn, ND]),
                    s0=-float(ND),
                )

                ot = pool.tile([P, gn], odt, tag=f"o{n}")
                red_eng = nc.gpsimd if i in pool_reduce else nc.vector
                red_eng.tensor_reduce(
                    out=ot[:],
                    in_=prod[:].rearrange("p (g k) -> p g k", k=ND),
                    axis=mybir.AxisListType.X,
                    op=mybir.AluOpType.add,
                )
                nc.sync.dma_start(out=o_d[:, goff:goff + gn], in_=ot[:])
    if not nc.is_finalized():
        nc.finalize()
    return nc


def build_bass(fd_p=FD_CORE, nt=0, ramp=2, tail_ramp=0, gp=0, lens_cast=0, sched=0, vb=4, ldma=1):
    """Build the per-core Bass program.

    fd_p: total fp32 elements per partition (divisible by nt*16)
    nt:   number of full-size tiles the bulk is split into
    ramp: split the first tile into `ramp` sub-tiles for a faster pipeline
          warm-up (0/1 = disabled)
    tail_ramp: split the last tile into `tail_ramp` sub-tiles so the final
          output DMA shrinks (0/1 = disabled)
    gp:   number of full-size tiles whose select is computed on GPSIMD via
          the stock mask pipeline (ACT expansion + eq + mult) instead of the
          DVE custom op, to offload the DVE bottleneck
    """
    import concourse.bacc as bacc
    import concourse.mybir as mybir
    import ml_dtypes
    from concourse.tile import TileContext

    f32 = mybir.dt.float32
    i32 = mybir.dt.int32
    bf16 = mybir.dt.bfloat16

    fdt = fd_p // nt if nt else fd_p
    assert fdt % ND == 0 and fdt * (nt or 1) == fd_p
    g_p = fd_p // ND

    # tile schedule: (elem offset, elems) per partition
    if nt == 0:
        # mixed schedule: small tiles to ramp the pipeline, 4096-elem tiles
        # for the overhead-amortized steady phase, 2048 tail
        if sched == 2:
            sizes = [1024] * 2 + [2048] * 2 + [4096] * 6 + [1024] + [512] * 2
        elif sched:
            sizes = [1024] * 2 + [2048] * 3 + [4096] * 5 + [2048] * 2
        else:
            sizes = [1024] * 2 + [2048] * 2 + [4096] * 6 + [2048]
        assert sum(sizes) == fd_p
        tiles, off0 = [], 0
        for s in sizes:
            tiles.append((off0, s))
            off0 += s
    else:
        tiles = [(t * fdt, fdt) for t in range(nt)]
        if ramp and ramp > 1 and fdt % (ramp * ND) == 0:
            sub = fdt // ramp
            tiles[0:1] = [(i * sub, sub) for i in range(ramp)]
        if tail_ramp and tail_ramp > 1 and fdt % (tail_ramp * ND) == 0:
            off0 = tiles[-1][0]
            sub = fdt // tail_ramp
            tiles[-1:] = [(off0 + i * sub, sub) for i in range(tail_ramp)]

    sel16 = _get_sel16()

    nc = bacc.Bacc("TRN2", target_bir_lowering=False)

    v_d = nc.dram_tensor("v", [P, fd_p], f32, kind="ExternalInput")
    l_d = nc.dram_tensor("lens", [P, g_p], i32, kind="ExternalInput")
    o_d = nc.dram_tensor("o", [P, g_p], f32, kind="ExternalOutput")

    # which tile indices run on GPSIMD: spread through the middle fulls
    full_idx = [i for i, (_, n) in enumerate(tiles) if n == fdt]
    gp_set = set(full_idx[1:1 + gp]) if gp else set()

    iota_d = None
    if gp_set:
        iota_np = np.tile(np.arange(1, ND + 1, dtype=np.float32), fdt // ND)
        iota_np = np.ascontiguousarray(
            iota_np.astype(ml_dtypes.bfloat16).reshape(1, fdt)
        )
        iota_d = nc.inline_tensor(iota_np, name="iota1_const")

    with TileContext(nc) as tc:
        with tc.tile_pool(name="work", bufs=3) as pool:
            if gp_set:
                iota_t = pool.tile([P, fdt], bf16, tag="iota", bufs=1)
                nc.gpsimd.dma_start(
                    out=iota_t[:], in_=iota_d[0:1, :].partition_broadcast(P)
                )
            for i, (off, n) in enumerate(tiles):
                goff, gn = off // ND, n // ND
                vt = pool.tile([P, n], f32, tag=f"v{n}", bufs=(vb if n == 4096 else 2 if n <= 1024 else 3))
                nc.sync.dma_start(out=vt[:], in_=v_d[:, off:off + n])
                lt = pool.tile([P, gn], i32, tag=f"l{n}")
                # ldma=1: lens via SWDGE so it never queues behind V tiles
                # on the HWDGE rings during the ramp
                (nc.gpsimd if ldma else nc.sync).dma_start(
                    out=lt[:], in_=l_d[:, goff:goff + gn])

                if lens_cast:
                    # int32 -> fp32 cast on the (otherwise idle) ACT engine
                    lf = pool.tile([P, gn], f32, tag=f"lf{n}")
                    nc.scalar.copy(out=lf[:], in_=lt[:])
                else:
                    # DVE read port converts int32 -> fp32 internally
                    lf = lt

                prod = pool.tile([P, n], f32, tag=f"prod{n}", bufs=2)
                if i in gp_set:
                    # GPSIMD pipeline: ACT expands lens to bf16, GPSIMD does
                    # eq + mult (frees the DVE for other tiles)
                    lexp = pool.tile([P, n], bf16, tag="lexp", bufs=2)
                    nc.scalar.copy(
                        out=lexp[:].rearrange("p (g k) -> p g k", k=ND),
                        in_=lf[:, :, None].to_broadcast([P, gn, ND]),
                    )
                    # eq on DVE (2x bf16), mult on GPSIMD (Pool rejects the
                    # BITVEC is_equal opcode but supports ARITH mult)
                    mask = pool.tile([P, n], bf16, tag="mask", bufs=2)
                    nc.vector.tensor_tensor(
                        out=mask[:], in0=lexp[:], in1=iota_t[:],
                        op=mybir.AluOpType.is_equal,
                    )
                    nc.gpsimd.tensor_tensor(
                        out=prod[:], in0=mask[:], in1=vt[:],
                        op=mybir.AluOpType.mult,
                    )
                else:
                    nc.vector._custom_dve(
                        sel16,
                        out=prod[:].rearrange("p (g k) -> p g k", k=ND),
                        in0=vt[:].rearrange("p (g k) -> p g k", k=ND),
                        in1=lf[:, :, None].to_broadcast([P, gn, ND]),
                        s0=-float(ND),
                    )

                ot = pool.tile([P, gn], f32, tag=f"o{n}")
                nc.vector.tensor_reduce(
                    out=ot[:],
                    in_=prod[:].rearrange("p (g k) -> p g k", k=ND),
                    axis=mybir.AxisListType.X,
                    op=mybir.AluOpType.add,
                )
                nc.sync.dma_start(out=o_d[:, goff:goff + gn], in_=ot[:])
    if not nc.is_finalized():
        nc.finalize()
    return nc


_CACHE = {}

# active build configuration: (builder_name, kwargs)
CONFIG = ("v2", {})


def _get_nc():
    name, kw = CONFIG
    key = (name, tuple(sorted(kw.items())))
    if key not in _CACHE:
        builder = {"v1": build_bass, "v2": build_v2, "v3": build_v3,
                   "v4": build_v4, "v5": build_v5}[name]
        _CACHE[key] = builder(**kw)
    return _CACHE[key]


def kernel(verified_id, accept_lens, num_draft_tokens, **run_kw):
    import ml_dtypes
    from concourse.bass_utils import run_bass_kernel_spmd

    name, bkw = CONFIG
    lens_i8 = (bool(bkw.get("lens_i8", 0)) and name == "v2") or (
        bool(bkw.get("lens_u8", 1)) and name in ("v3", "v4", "v5"))
    out_bf16 = bool(bkw.get("out_bf16", name in ("v3", "v4", "v5")))
    if name in ("v4", "v5") and bkw.get("out_f32", 0):
        out_bf16 = False

    assert int(num_draft_tokens) == ND
    v = np.ascontiguousarray(np.asarray(verified_id, dtype=np.float32))
    lens = np.asarray(accept_lens)
    assert v.shape == (BS * ND,) and lens.shape == (BS,)
    ldt = np.uint8 if lens_i8 else np.int32
    l32 = np.ascontiguousarray(lens.astype(ldt))

    v3 = v.reshape(N_CORES, P, FD_CORE)
    if name == "v5":
        v3 = v5_host_transpose(v3, bkw.get("sched", 2))
    l3 = l32.reshape(N_CORES, P, G_P)

    nc = _get_nc()
    in_maps = [{"v": v3[c], "lens": l3[c]} for c in range(N_CORES)]
    res = run_bass_kernel_spmd(nc, in_maps, core_ids=list(range(N_CORES)), **run_kw)
    out = np.stack([res.results[c]["o"] for c in range(N_CORES)])
    if out_bf16:
        out = out.view(ml_dtypes.bfloat16).astype(np.float32)
    ret = out.reshape(-1).astype(np.float32, copy=False)
    if run_kw:
        return ret, res
    return ret

